# revision 11
# baseline (speedup 1.0000x reference)
"""Trainium2 Bass kernel for nn_Attention_75806172775136 (topk_masking).

Data-parallel over 8 NeuronCores: 8 samples per core, weights replicated.
Reference computes, per sample: qkv proj -> attn logits -> CLS-token top-138
mask -> masked softmax -> attn @ v -> out proj; returns (out, keep_mask,
attn_rt).

Per-core pipeline (layouts transposed so contraction rides partitions; host
pre-transposes inputs / post-transposes outputs):
  1. qkT GEMM (fp32r, N=394 sample-pairs): psum += Wqk @ xT, bias via K=1
     matmul; copied out twice: bf16 (attention) + f32 k-chunks/q0 (exact cls)
  2. v GEMM (fp32r) in natural [n, o] layout -> bf16
  3. cls scores: fp32 matmuls q0 . kT  (exact top-k selection vs reference)
  4. top-138 via vector.max + match_replace (exact-K, matches jax top_k)
  5. S^T = kT.T @ qT per (sample, head) in bf16; raw logits -> attn_rt
  6. e^T = exp(S^T) (no max-shift: |S| < 40); AV with lhsT = [v*keep | keep]
     -> out2T rows 0-63 numerator, row 64 denominator
  7. normalize: reciprocal -> gpsimd partition_broadcast -> DVE mul -> A^T
  8. out^T = Wp @ A^T + b (fp32r, N=394)
"""

import os
import sys

sys.path.insert(0, "/opt/trn_rl_repo")

import numpy as np

import concourse.bass as bass
import concourse.bacc as bacc
import concourse.mybir as mybir
from concourse.tile import TileContext
from concourse.bass_utils import run_bass_kernel_spmd

F32 = mybir.dt.float32
F32R = mybir.dt.float32r
BF16 = mybir.dt.bfloat16
AF = mybir.ActivationFunctionType
OP = mybir.AluOpType

NCORES = 8
B_CORE = 8
N = 197
C = 768
H = 12
HD = 64
KEEP = 138
NEG = -1.0e9

SPH = 2              # samples per quarter
NH = SPH * N         # 394
NPAIR = 2 * N        # 394


def _ts(i, s):
    return slice(i * s, (i + 1) * s)


def build_nc():
    nc = bacc.Bacc("TRN2", target_bir_lowering=False, debug=False)

    xT_d = nc.declare_dram_parameter("xT", [C, B_CORE * N], F32, isOutput=False)
    wqkvT_d = nc.declare_dram_parameter("wqkvT", [C, 3 * C], F32, isOutput=False)
    bqk_d = nc.declare_dram_parameter("bqk", [2 * C], F32, isOutput=False)
    bv_d = nc.declare_dram_parameter("bv", [C], F32, isOutput=False)
    wpT_d = nc.declare_dram_parameter("wpT", [C, C], F32, isOutput=False)
    bp_d = nc.declare_dram_parameter("bp", [C], F32, isOutput=False)
    ones_d = nc.declare_dram_parameter("ones", [NPAIR], F32, isOutput=False)

    outT_d = nc.declare_dram_parameter("outT", [C, B_CORE * N], F32, isOutput=True)
    keep_d = nc.declare_dram_parameter("keep", [B_CORE, N], F32, isOutput=True)
    # [b, h, k, q] -- host transposes the last two dims
    art_d = nc.declare_dram_parameter("attn_rt_t", [B_CORE, H, N, N], F32, isOutput=True)

    NCHAIN = SPH * H  # chains per quarter

    from contextlib import ExitStack

    with ExitStack() as ctx:
        tc = ctx.enter_context(TileContext(nc))
        ctx.enter_context(nc.allow_low_precision(reason="bf16/f32r compute by design"))
        cpool = ctx.enter_context(tc.tile_pool(name="const", bufs=1))
        wpool = ctx.enter_context(tc.tile_pool(name="wq", bufs=1))
        xpool = ctx.enter_context(tc.tile_pool(name="xh", bufs=2))
        qkpool = ctx.enter_context(tc.tile_pool(name="qk", bufs=2))
        vpool = ctx.enter_context(tc.tile_pool(name="vv", bufs=4))
        v2pool = ctx.enter_context(tc.tile_pool(name="v2", bufs=2))
        apool = ctx.enter_context(tc.tile_pool(name="att", bufs=3))
        npool = ctx.enter_context(tc.tile_pool(name="nrm", bufs=2))
        upool12 = ctx.enter_context(tc.tile_pool(name="usb12", bufs=12))
        scpool = ctx.enter_context(tc.tile_pool(name="sc", bufs=1))
        atpool = ctx.enter_context(tc.tile_pool(name="aT", bufs=2))
        opool = ctx.enter_context(tc.tile_pool(name="oT", bufs=2))
        ps_mm = ctx.enter_context(tc.tile_pool(name="ps_mm", bufs=2, space="PSUM"))
        ps_s = ctx.enter_context(tc.tile_pool(name="ps_s", bufs=2, space="PSUM"))
        ps_av = ctx.enter_context(tc.tile_pool(name="ps_av", bufs=3, space="PSUM"))
        ps_cls = ctx.enter_context(tc.tile_pool(name="ps_cls", bufs=1, space="PSUM"))
        if True:
            # ---- weights / constants (persistent) ----
            w_sb = []
            for ci in range(6):
                t = wpool.tile([128, 3 * C], F32R, tag=f"wqkv{ci}")
                nc.sync.dma_start(out=t[:, :], in_=wqkvT_d[_ts(ci, 128), :].bitcast(F32R))
                w_sb.append(t)
            wp_sb = []
            for ci in range(6):
                t = wpool.tile([128, C], F32R, tag=f"wp{ci}")
                nc.sync.dma_start(out=t[:, :], in_=wpT_d[_ts(ci, 128), :].bitcast(F32R))
                wp_sb.append(t)
            bqk_row = cpool.tile([1, 2 * C], F32R)
            nc.sync.dma_start(out=bqk_row[:, :], in_=bqk_d[:].rearrange("(a c) -> a c", a=1).bitcast(F32R))
            bv_row = cpool.tile([1, C], F32R)
            nc.sync.dma_start(out=bv_row[:, :], in_=bv_d[:].rearrange("(a c) -> a c", a=1).bitcast(F32R))
            bp_row = cpool.tile([1, C], F32R)
            nc.sync.dma_start(out=bp_row[:, :], in_=bp_d[:].rearrange("(a c) -> a c", a=1).bitcast(F32R))
            ones_r = cpool.tile([1, NPAIR], F32R)
            nc.sync.dma_start(out=ones_r[:, :], in_=ones_d[:].rearrange("(a c) -> a c", a=1).bitcast(F32R))

            for q in range(4):
                b0 = q * SPH
                xh = xpool.tile([128, 6, NH], F32R, tag="xT")
                for ci in range(6):
                    nc.sync.dma_start(
                        out=xh[:, ci, :],
                        in_=xT_d[_ts(ci, 128), b0 * N : (b0 + SPH) * N].bitcast(F32R),
                    )

                # ---- qk GEMM (one sample-pair, N=394) ----
                qkb = qkpool.tile([128, 12, NH], BF16, tag="qkb")
                ktf = qkpool.tile([128, 6, NH], F32, tag="ktf")
                q0f = qkpool.tile([128, 6, SPH], F32, tag="q0f")
                for oc in range(12):
                    ps = ps_mm.tile([128, 512], F32, tag="mm")
                    for ci in range(6):
                        nc.tensor.matmul(
                            ps[:, :NH],
                            lhsT=w_sb[ci][:, _ts(oc, 128)],
                            rhs=xh[:, ci, :],
                            start=(ci == 0),
                            stop=False,
                        )
                    nc.tensor.matmul(
                        ps[:, :NH],
                        lhsT=bqk_row[:, _ts(oc, 128)],
                        rhs=ones_r[:, :],
                        start=False,
                        stop=True,
                    )
                    nc.scalar.copy(out=qkb[:, oc, :], in_=ps[:, :NH])
                    if oc >= 6:
                        nc.vector.tensor_copy(out=ktf[:, oc - 6, :], in_=ps[:, :NH])
                    else:
                        for sb_ in range(SPH):
                            nc.vector.tensor_copy(
                                out=q0f[:, oc, sb_ : sb_ + 1],
                                in_=ps[:, sb_ * N : sb_ * N + 1],
                            )

                # ---- cls scores (fp32 exact) + gather ----
                sc4 = scpool.tile([SPH, N], F32, tag="sc4")
                for bl in range(SPH):
                    ps = ps_cls.tile([1, N], F32, tag="cls")
                    for ci in range(6):
                        nc.tensor.matmul(
                            ps[:, :],
                            lhsT=q0f[:, ci, bl : bl + 1],
                            rhs=ktf[:, ci, bl * N : bl * N + N],
                            start=(ci == 0),
                            stop=(ci == 5),
                        )
                    row = scpool.tile([1, N], F32, tag=f"scrow{bl}")
                    nc.vector.tensor_copy(out=row[:, :], in_=ps[:, :])
                    nc.sync.dma_start(out=sc4[bl : bl + 1, :], in_=row[:, :])

                # ---- v GEMM (after cls: fills the top-k PE gap) ----
                v_sb = []
                for bl in range(SPH):
                    vt = vpool.tile([128, 2, C], BF16, tag="v")
                    for chk, (r0, rn) in enumerate(((0, 128), (128, 69))):
                        pss = [ps_mm.tile([128, 512], F32, tag="mm", name=f"psv{_}") for _ in range(2)]
                        for ci in range(6):
                            for ic, (c0, cn) in enumerate(((0, 512), (512, 256))):
                                nc.tensor.matmul(
                                    pss[ic][:rn, :cn],
                                    lhsT=xh[:, ci, bl * N + r0 : bl * N + r0 + rn],
                                    rhs=w_sb[ci][:, 2 * C + c0 : 2 * C + c0 + cn],
                                    start=(ci == 0),
                                    stop=False,
                                )
                        for ic, (c0, cn) in enumerate(((0, 512), (512, 256))):
                            nc.tensor.matmul(
                                pss[ic][:rn, :cn],
                                lhsT=ones_r[:, :rn],
                                rhs=bv_row[:, c0 : c0 + cn],
                                start=False,
                                stop=True,
                            )
                            nc.scalar.copy(out=vt[:rn, chk, c0 : c0 + cn], in_=pss[ic][:rn, :cn])
                    v_sb.append(vt)

                # ---- top-K scan ----
                work = scpool.tile([SPH, N], F32, tag="work")
                nc.vector.tensor_copy(out=work[:, :], in_=sc4[:, :])
                nc.vector.memset(work[:, 0:1], -2.0e9)
                m8 = scpool.tile([SPH, 8], F32, tag="m8")
                left = KEEP
                while left > 0:
                    nc.vector.max(out=m8[:, :], in_=work[:, :])
                    if left < 8:
                        nc.vector.memset(m8[:, left:], NEG)
                    nc.vector.match_replace(
                        out=work[:, :], in_to_replace=m8[:, :],
                        in_values=work[:, :], imm_value=NEG,
                    )
                    left -= 8
                keep4 = scpool.tile([SPH, N], F32, tag="keep4")
                nc.vector.tensor_tensor(out=keep4[:, :], in0=work[:, :], in1=sc4[:, :], op=OP.not_equal)
                nc.sync.dma_start(out=keep_d[b0 : b0 + SPH, :], in_=keep4[:, :])

                kT1 = scpool.tile([128, SPH], F32, tag="kT1")
                kT2 = scpool.tile([128, SPH], F32, tag="kT2")
                for bl in range(SPH):
                    nc.sync.dma_start(out=kT1[:, bl : bl + 1], in_=keep4[bl : bl + 1, 0:128])
                    nc.sync.dma_start(out=kT2[0:69, bl : bl + 1], in_=keep4[bl : bl + 1, 128:197])

                # ---- V2 = [v*keep | keep] bf16 ----
                v2_sb = []
                for bl in range(SPH):
                    v2 = v2pool.tile([128, 2, H, HD + 1], BF16, tag="v2")
                    for chk, (kTc, rn) in enumerate(((kT1, 128), (kT2, 69))):
                        nc.vector.tensor_scalar(
                            out=v2[:rn, chk, :, 0:HD],
                            in0=v_sb[bl][:rn, chk, :].rearrange("p (h d) -> p h d", h=H),
                            scalar1=kTc[:rn, bl : bl + 1],
                            scalar2=None,
                            op0=OP.mult,
                        )
                        nc.vector.tensor_copy(
                            out=v2[:rn, chk, :, HD],
                            in_=kTc[:rn, bl : bl + 1].to_broadcast([rn, H]),
                        )
                    v2_sb.append(v2)

                # ---- attention chains (normalize batched per sample) ----
                aT = atpool.tile([128, 6, NH], F32R, tag="aT")
                for bl in range(SPH):
                    b = b0 + bl
                    D = npool.tile([H, N], F32, tag="D")
                    R = npool.tile([H, N], BF16, tag="R")
                    usbs = []
                    for h in range(H):
                        oc, po = h // 2, (h % 2) * HD
                        ps = ps_s.tile([128, NPAIR], F32, tag="sT")
                        nc.tensor.matmul(
                            ps[:, 0:N],
                            lhsT=qkb[po : po + HD, 6 + oc, bl * N : bl * N + 128],
                            rhs=qkb[po : po + HD, oc, bl * N : bl * N + N],
                            start=True,
                            stop=True,
                        )
                        nc.tensor.matmul(
                            ps[0:69, N : 2 * N],
                            lhsT=qkb[po : po + HD, 6 + oc, bl * N + 128 : bl * N + N],
                            rhs=qkb[po : po + HD, oc, bl * N : bl * N + N],
                            start=True,
                            stop=True,
                        )
                        st = apool.tile([128, NPAIR], F32, tag="sT_sb")
                        if h % 2 == 0:
                            nc.scalar.copy(out=st[:, :], in_=ps[:, :])
                        else:
                            nc.vector.tensor_copy(out=st[:, :], in_=ps[:, :])
                        nc.sync.dma_start(out=art_d[b, h, 0:128, :], in_=st[:, 0:N])
                        nc.sync.dma_start(out=art_d[b, h, 128:N, :], in_=st[0:69, N : 2 * N])
                        et = apool.tile([128, NPAIR], BF16, tag="eT")
                        nc.scalar.activation(out=et[:, :], in_=ps[:, :], func=AF.Exp)

                        ps_o = ps_av.tile([HD + 1, N], F32, tag="av")
                        nc.tensor.matmul(
                            ps_o[:, :],
                            lhsT=v2_sb[bl][0:128, 0, h, :],
                            rhs=et[:, 0:N],
                            start=True,
                            stop=False,
                        )
                        nc.tensor.matmul(
                            ps_o[:, :],
                            lhsT=v2_sb[bl][0:69, 1, h, :],
                            rhs=et[0:69, N : 2 * N],
                            start=False,
                            stop=True,
                        )
                        drow = apool.tile([1, N], F32, tag="drow")
                        nc.scalar.copy(out=drow[:, :], in_=ps_o[HD : HD + 1, :])
                        nc.sync.dma_start(out=D[h : h + 1, :], in_=drow[:, :])
                        usb = upool12.tile([HD, N], BF16, tag="usb")
                        nc.vector.tensor_copy(out=usb[:, :], in_=ps_o[0:HD, :])
                        usbs.append(usb)

                    nc.vector.reciprocal(out=R[:, :], in_=D[:, :])
                    for h in range(H):
                        oc, po = h // 2, (h % 2) * HD
                        rb = npool.tile([1, N], BF16, tag="rb")
                        nc.sync.dma_start(out=rb[:, :], in_=R[h : h + 1, :])
                        bc = npool.tile([HD, N], BF16, tag="bc")
                        nc.gpsimd.partition_broadcast(bc[:, :], rb[:, :])
                        nc.vector.tensor_tensor(
                            out=aT[po : po + HD, oc, bl * N : bl * N + N],
                            in0=usbs[h][:, :],
                            in1=bc[:, :],
                            op=OP.mult,
                        )

                # ---- proj ----
                for oc in range(6):
                    ps = ps_mm.tile([128, 512], F32, tag="mm")
                    for ci in range(6):
                        nc.tensor.matmul(
                            ps[:, :NPAIR],
                            lhsT=wp_sb[ci][:, _ts(oc, 128)],
                            rhs=aT[:, ci, :],
                            start=(ci == 0),
                            stop=False,
                        )
                    nc.tensor.matmul(
                        ps[:, :NPAIR],
                        lhsT=bp_row[:, _ts(oc, 128)],
                        rhs=ones_r[:, :],
                        start=False,
                        stop=True,
                    )
                    ot = opool.tile([128, NPAIR], F32, tag="oT")
                    nc.scalar.copy(out=ot[:, :], in_=ps[:, :NPAIR])
                    nc.sync.dma_start(
                        out=outT_d[_ts(oc, 128), b0 * N : (b0 + SPH) * N],
                        in_=ot[:, :],
                    )
    nc.finalize()
    return nc


_NC_CACHE = None
LAST_RESULT = None


def kernel(x, qkv_w, qkv_b, proj_w, proj_b, num_keep_node):
    global _NC_CACHE
    assert int(num_keep_node) == KEEP
    x = np.asarray(x, np.float32)
    scale = float(HD) ** -0.5

    wqkvT = np.ascontiguousarray(np.asarray(qkv_w, np.float32).T)  # [768, 2304]
    wqkvT[:, :C] = wqkvT[:, :C] * scale
    bqk = np.asarray(qkv_b, np.float32)[: 2 * C].copy()
    bqk[:C] *= scale
    bv = np.ascontiguousarray(np.asarray(qkv_b, np.float32)[2 * C :])
    wpT = np.ascontiguousarray(np.asarray(proj_w, np.float32).T)
    bp = np.ascontiguousarray(np.asarray(proj_b, np.float32))
    ones = np.ones(NPAIR, np.float32)

    if _NC_CACHE is None:
        _NC_CACHE = build_nc()
    nc = _NC_CACHE

    in_maps = []
    for c in range(NCORES):
        xs = x[c * B_CORE : (c + 1) * B_CORE].reshape(B_CORE * N, C)
        in_maps.append(
            {
                "xT": np.ascontiguousarray(xs.T),
                "wqkvT": wqkvT,
                "bqk": bqk,
                "bv": bv,
                "wpT": wpT,
                "bp": bp,
                "ones": ones,
            }
        )
    global LAST_RESULT
    LAST_RESULT = run_bass_kernel_spmd(
        nc,
        in_maps,
        core_ids=list(range(NCORES)),
        trace=bool(os.environ.get("KTRACE")),
        tmpdir=os.environ.get("KTRACE_DIR") or None,
    )
    res = LAST_RESULT.results

    outs, keeps, arts = [], [], []
    for r in res:
        outs.append(np.ascontiguousarray(r["outT"].reshape(C, B_CORE, N).transpose(1, 2, 0)))
        keeps.append(r["keep"][:, :, None])
        arts.append(np.ascontiguousarray(r["attn_rt_t"].transpose(0, 1, 3, 2)))
    return (
        np.concatenate(outs, 0),
        np.concatenate(keeps, 0),
        np.concatenate(arts, 0),
    )


# revision 12
# speedup vs baseline: 1.0437x; 1.0437x over previous
"""Trainium2 Bass kernel for nn_Attention_75806172775136 (topk_masking).

Data-parallel over 8 NeuronCores: 8 samples per core, weights replicated.
Reference computes, per sample: qkv proj -> attn logits -> CLS-token top-138
mask -> masked softmax -> attn @ v -> out proj; returns (out, keep_mask,
attn_rt).

Per-core pipeline (layouts transposed so contraction rides partitions; host
pre-transposes inputs / post-transposes outputs):
  1. qkT GEMM (fp32r, N=394 sample-pairs): psum += Wqk @ xT, bias via K=1
     matmul; copied out twice: bf16 (attention) + f32 k-chunks/q0 (exact cls)
  2. v GEMM (fp32r) in natural [n, o] layout -> bf16
  3. cls scores: fp32 matmuls q0 . kT  (exact top-k selection vs reference)
  4. top-138 via vector.max + match_replace (exact-K, matches jax top_k)
  5. S^T = kT.T @ qT per (sample, head) in bf16; raw logits -> attn_rt
  6. e^T = exp(S^T) (no max-shift: |S| < 40); AV with lhsT = [v*keep | keep]
     -> out2T rows 0-63 numerator, row 64 denominator
  7. normalize: reciprocal -> gpsimd partition_broadcast -> DVE mul -> A^T
  8. out^T = Wp @ A^T + b (fp32r, N=394)
"""

import os
import sys

sys.path.insert(0, "/opt/trn_rl_repo")

import numpy as np

import concourse.bass as bass
import concourse.bacc as bacc
import concourse.mybir as mybir
from concourse.tile import TileContext
from concourse.bass_utils import run_bass_kernel_spmd

F32 = mybir.dt.float32
F32R = mybir.dt.float32r
BF16 = mybir.dt.bfloat16
AF = mybir.ActivationFunctionType
OP = mybir.AluOpType

NCORES = 8
B_CORE = 8
N = 197
C = 768
H = 12
HD = 64
KEEP = 138
NEG = -1.0e9

SPH = 2              # samples per quarter
NH = SPH * N         # 394
NPAIR = 2 * N        # 394


def _ts(i, s):
    return slice(i * s, (i + 1) * s)


def build_nc():
    nc = bacc.Bacc("TRN2", target_bir_lowering=False, debug=False)

    xT_d = nc.declare_dram_parameter("xT", [C, B_CORE * N], F32, isOutput=False)
    wqkvT_d = nc.declare_dram_parameter("wqkvT", [C, 3 * C], F32, isOutput=False)
    bqk_d = nc.declare_dram_parameter("bqk", [2 * C], F32, isOutput=False)
    bv_d = nc.declare_dram_parameter("bv", [C], F32, isOutput=False)
    wpT_d = nc.declare_dram_parameter("wpT", [C, C], F32, isOutput=False)
    bp_d = nc.declare_dram_parameter("bp", [C], F32, isOutput=False)
    ones_d = nc.declare_dram_parameter("ones", [NPAIR], F32, isOutput=False)

    outT_d = nc.declare_dram_parameter("outT", [C, B_CORE * N], F32, isOutput=True)
    keep_d = nc.declare_dram_parameter("keep", [B_CORE, N], F32, isOutput=True)
    # [b, h, k, q] -- host transposes the last two dims
    art_d = nc.declare_dram_parameter("attn_rt_t", [B_CORE, H, N, N], F32, isOutput=True)

    NCHAIN = SPH * H  # chains per quarter

    from contextlib import ExitStack

    with ExitStack() as ctx:
        tc = ctx.enter_context(TileContext(nc))
        ctx.enter_context(nc.allow_low_precision(reason="bf16/f32r compute by design"))
        cpool = ctx.enter_context(tc.tile_pool(name="const", bufs=1))
        wpool = ctx.enter_context(tc.tile_pool(name="wq", bufs=1))
        xpool = ctx.enter_context(tc.tile_pool(name="xh", bufs=2))
        qkpool = ctx.enter_context(tc.tile_pool(name="qk", bufs=2))
        vpool = ctx.enter_context(tc.tile_pool(name="vv", bufs=4))
        v2pool = ctx.enter_context(tc.tile_pool(name="v2", bufs=2))
        apool = ctx.enter_context(tc.tile_pool(name="att", bufs=3))
        npool = ctx.enter_context(tc.tile_pool(name="nrm", bufs=2))
        upool12 = ctx.enter_context(tc.tile_pool(name="usb12", bufs=12))
        scpool = ctx.enter_context(tc.tile_pool(name="sc", bufs=1))
        atpool = ctx.enter_context(tc.tile_pool(name="aT", bufs=2))
        opool = ctx.enter_context(tc.tile_pool(name="oT", bufs=2))
        ps_mm = ctx.enter_context(tc.tile_pool(name="ps_mm", bufs=2, space="PSUM"))
        ps_s = ctx.enter_context(tc.tile_pool(name="ps_s", bufs=2, space="PSUM"))
        ps_av = ctx.enter_context(tc.tile_pool(name="ps_av", bufs=3, space="PSUM"))
        ps_cls = ctx.enter_context(tc.tile_pool(name="ps_cls", bufs=1, space="PSUM"))
        if True:
            # ---- weights / constants (persistent) ----
            w_sb = []
            for ci in range(6):
                t = wpool.tile([128, 3 * C], F32R, tag=f"wqkv{ci}")
                nc.sync.dma_start(out=t[:, :], in_=wqkvT_d[_ts(ci, 128), :].bitcast(F32R))
                w_sb.append(t)
            wp_sb = []
            for ci in range(6):
                t = wpool.tile([128, C], F32R, tag=f"wp{ci}")
                nc.sync.dma_start(out=t[:, :], in_=wpT_d[_ts(ci, 128), :].bitcast(F32R))
                wp_sb.append(t)
            bqk_col = cpool.tile([128, 12], F32)
            nc.sync.dma_start(out=bqk_col[:, :], in_=bqk_d[:].rearrange("(c p) -> p c", p=128))
            bv_row = cpool.tile([1, C], F32R)
            nc.sync.dma_start(out=bv_row[:, :], in_=bv_d[:].rearrange("(a c) -> a c", a=1).bitcast(F32R))
            bp_col = cpool.tile([128, 6], F32)
            nc.sync.dma_start(out=bp_col[:, :], in_=bp_d[:].rearrange("(c p) -> p c", p=128))
            ones_r = cpool.tile([1, NPAIR], F32R)
            nc.sync.dma_start(out=ones_r[:, :], in_=ones_d[:].rearrange("(a c) -> a c", a=1).bitcast(F32R))
            ones_b = cpool.tile([1, 1], BF16)
            nc.vector.memset(ones_b[:, :], 1.0)

            for q in range(4):
                b0 = q * SPH
                xh = xpool.tile([128, 6, NH], F32R, tag="xT")
                for ci in range(6):
                    nc.sync.dma_start(
                        out=xh[:, ci, :],
                        in_=xT_d[_ts(ci, 128), b0 * N : (b0 + SPH) * N].bitcast(F32R),
                    )

                # ---- qk GEMM (one sample-pair, N=394) ----
                qkb = qkpool.tile([128, 12, NH], BF16, tag="qkb")
                ktf = qkpool.tile([128, 6, NH], F32, tag="ktf")
                q0f = qkpool.tile([128, 6, SPH], F32, tag="q0f")
                for oc in range(12):
                    ps = ps_mm.tile([128, 512], F32, tag="mm")
                    for ci in range(6):
                        nc.tensor.matmul(
                            ps[:, :NH],
                            lhsT=w_sb[ci][:, _ts(oc, 128)],
                            rhs=xh[:, ci, :],
                            start=(ci == 0),
                            stop=(ci == 5),
                        )
                    nc.vector.tensor_scalar(
                        out=qkb[:, oc, :], in0=ps[:, :NH],
                        scalar1=bqk_col[:, oc : oc + 1], scalar2=None, op0=OP.add,
                    )
                    if oc >= 6:
                        nc.vector.tensor_scalar(
                            out=ktf[:, oc - 6, :], in0=ps[:, :NH],
                            scalar1=bqk_col[:, oc : oc + 1], scalar2=None, op0=OP.add,
                        )
                    else:
                        for sb_ in range(SPH):
                            nc.vector.tensor_scalar(
                                out=q0f[:, oc, sb_ : sb_ + 1],
                                in0=ps[:, sb_ * N : sb_ * N + 1],
                                scalar1=bqk_col[:, oc : oc + 1], scalar2=None, op0=OP.add,
                            )

                # ---- cls scores (fp32 exact) + gather ----
                sc4 = scpool.tile([SPH, N], F32, tag="sc4")
                for bl in range(SPH):
                    ps = ps_cls.tile([1, N], F32, tag="cls")
                    for ci in range(6):
                        nc.tensor.matmul(
                            ps[:, :],
                            lhsT=q0f[:, ci, bl : bl + 1],
                            rhs=ktf[:, ci, bl * N : bl * N + N],
                            start=(ci == 0),
                            stop=(ci == 5),
                        )
                    row = scpool.tile([1, N], F32, tag=f"scrow{bl}")
                    nc.vector.tensor_copy(out=row[:, :], in_=ps[:, :])
                    nc.gpsimd.dma_start(out=sc4[bl : bl + 1, :], in_=row[:, :])

                # ---- v GEMM (after cls: fills the top-k PE gap) ----
                v_sb = []
                for bl in range(SPH):
                    vt = vpool.tile([128, 2, C], BF16, tag="v")
                    for chk, (r0, rn) in enumerate(((0, 128), (128, 69))):
                        pss = [ps_mm.tile([128, 512], F32, tag="mm", name=f"psv{_}") for _ in range(2)]
                        for ci in range(6):
                            for ic, (c0, cn) in enumerate(((0, 512), (512, 256))):
                                nc.tensor.matmul(
                                    pss[ic][:rn, :cn],
                                    lhsT=xh[:, ci, bl * N + r0 : bl * N + r0 + rn],
                                    rhs=w_sb[ci][:, 2 * C + c0 : 2 * C + c0 + cn],
                                    start=(ci == 0),
                                    stop=False,
                                )
                        for ic, (c0, cn) in enumerate(((0, 512), (512, 256))):
                            nc.tensor.matmul(
                                pss[ic][:rn, :cn],
                                lhsT=ones_r[:, :rn],
                                rhs=bv_row[:, c0 : c0 + cn],
                                start=False,
                                stop=True,
                            )
                            nc.scalar.copy(out=vt[:rn, chk, c0 : c0 + cn], in_=pss[ic][:rn, :cn])
                    v_sb.append(vt)

                # ---- top-K scan ----
                work = scpool.tile([SPH, N], F32, tag="work")
                nc.vector.tensor_copy(out=work[:, :], in_=sc4[:, :])
                nc.vector.memset(work[:, 0:1], -2.0e9)
                m8 = scpool.tile([SPH, 8], F32, tag="m8")
                left = KEEP
                while left > 0:
                    nc.vector.max(out=m8[:, :], in_=work[:, :])
                    if left < 8:
                        nc.vector.memset(m8[:, left:], NEG)
                    nc.vector.match_replace(
                        out=work[:, :], in_to_replace=m8[:, :],
                        in_values=work[:, :], imm_value=NEG,
                    )
                    left -= 8
                keep4 = scpool.tile([SPH, N], F32, tag="keep4")
                nc.vector.tensor_tensor(out=keep4[:, :], in0=work[:, :], in1=sc4[:, :], op=OP.not_equal)
                nc.sync.dma_start(out=keep_d[b0 : b0 + SPH, :], in_=keep4[:, :])

                kT1 = scpool.tile([128, SPH], F32, tag="kT1")
                kT2 = scpool.tile([128, SPH], F32, tag="kT2")
                for bl in range(SPH):
                    nc.gpsimd.dma_start(out=kT1[:, bl : bl + 1], in_=keep4[bl : bl + 1, 0:128])
                    nc.gpsimd.dma_start(out=kT2[0:69, bl : bl + 1], in_=keep4[bl : bl + 1, 128:197])

                # ---- V2 = [v*keep | keep] bf16 ----
                v2_sb = []
                for bl in range(SPH):
                    v2 = v2pool.tile([128, 2, H, HD + 1], BF16, tag="v2")
                    for chk, (kTc, rn) in enumerate(((kT1, 128), (kT2, 69))):
                        nc.vector.tensor_scalar(
                            out=v2[:rn, chk, :, 0:HD],
                            in0=v_sb[bl][:rn, chk, :].rearrange("p (h d) -> p h d", h=H),
                            scalar1=kTc[:rn, bl : bl + 1],
                            scalar2=None,
                            op0=OP.mult,
                        )
                        nc.vector.tensor_copy(
                            out=v2[:rn, chk, :, HD],
                            in_=kTc[:rn, bl : bl + 1].to_broadcast([rn, H]),
                        )
                    v2_sb.append(v2)

                # ---- attention chains (normalize batched per sample) ----
                aT = atpool.tile([128, 6, NH], F32R, tag="aT")
                for bl in range(SPH):
                    b = b0 + bl
                    D = npool.tile([H, N], F32, tag="D")
                    R = npool.tile([H, N], BF16, tag="R")
                    usbs = []
                    for h in range(H):
                        oc, po = h // 2, (h % 2) * HD
                        ps = ps_s.tile([128, NPAIR], F32, tag="sT")
                        nc.tensor.matmul(
                            ps[:, 0:N],
                            lhsT=qkb[po : po + HD, 6 + oc, bl * N : bl * N + 128],
                            rhs=qkb[po : po + HD, oc, bl * N : bl * N + N],
                            start=True,
                            stop=True,
                        )
                        nc.tensor.matmul(
                            ps[0:69, N : 2 * N],
                            lhsT=qkb[po : po + HD, 6 + oc, bl * N + 128 : bl * N + N],
                            rhs=qkb[po : po + HD, oc, bl * N : bl * N + N],
                            start=True,
                            stop=True,
                        )
                        st = apool.tile([128, NPAIR], F32, tag="sT_sb")
                        if h % 2 == 0:
                            nc.scalar.copy(out=st[:, :], in_=ps[:, :])
                        else:
                            nc.vector.tensor_copy(out=st[:, :], in_=ps[:, :])
                        nc.sync.dma_start(out=art_d[b, h, 0:128, :], in_=st[:, 0:N])
                        nc.gpsimd.dma_start(out=art_d[b, h, 128:N, :], in_=st[0:69, N : 2 * N])
                        et = apool.tile([128, NPAIR], BF16, tag="eT")
                        nc.scalar.activation(out=et[:, :], in_=ps[:, :], func=AF.Exp)

                        ps_o = ps_av.tile([HD + 1, N], F32, tag="av")
                        nc.tensor.matmul(
                            ps_o[:, :],
                            lhsT=v2_sb[bl][0:128, 0, h, :],
                            rhs=et[:, 0:N],
                            start=True,
                            stop=False,
                        )
                        nc.tensor.matmul(
                            ps_o[:, :],
                            lhsT=v2_sb[bl][0:69, 1, h, :],
                            rhs=et[0:69, N : 2 * N],
                            start=False,
                            stop=True,
                        )
                        ps_j = ps_cls.tile([1, 64], F32, tag="cls", name="psj")
                        nc.tensor.matmul(ps_j[:, :], lhsT=ones_b[:, :], rhs=et[0:1, 0:HD], start=True, stop=True)
                        drow = apool.tile([1, N], F32, tag="drow")
                        nc.scalar.copy(out=drow[:, :], in_=ps_o[HD : HD + 1, :])
                        nc.gpsimd.dma_start(out=D[h : h + 1, :], in_=drow[:, :])
                        usb = upool12.tile([HD, N], BF16, tag="usb")
                        nc.vector.tensor_copy(out=usb[:, :], in_=ps_o[0:HD, :])
                        usbs.append(usb)

                    nc.vector.reciprocal(out=R[:, :], in_=D[:, :])
                    for h in range(H):
                        oc, po = h // 2, (h % 2) * HD
                        rb = npool.tile([1, N], BF16, tag="rb")
                        nc.gpsimd.dma_start(out=rb[:, :], in_=R[h : h + 1, :])
                        bc = npool.tile([HD, N], BF16, tag="bc")
                        nc.gpsimd.partition_broadcast(bc[:, :], rb[:, :])
                        ps_j2 = ps_cls.tile([1, 64], F32, tag="cls", name="psj2")
                        nc.tensor.matmul(ps_j2[:, :], lhsT=ones_b[:, :], rhs=bc[0:1, 0:HD], start=True, stop=True)
                        nc.vector.tensor_tensor(
                            out=aT[po : po + HD, oc, bl * N : bl * N + N],
                            in0=usbs[h][:, :],
                            in1=bc[:, :],
                            op=OP.mult,
                        )

                # ---- proj ----
                for oc in range(6):
                    ps = ps_mm.tile([128, 512], F32, tag="mm")
                    for ci in range(6):
                        nc.tensor.matmul(
                            ps[:, :NPAIR],
                            lhsT=wp_sb[ci][:, _ts(oc, 128)],
                            rhs=aT[:, ci, :],
                            start=(ci == 0),
                            stop=(ci == 5),
                        )
                    ot = opool.tile([128, NPAIR], F32, tag="oT")
                    nc.vector.tensor_scalar(
                        out=ot[:, :], in0=ps[:, :NPAIR],
                        scalar1=bp_col[:, oc : oc + 1], scalar2=None, op0=OP.add,
                    )
                    nc.sync.dma_start(
                        out=outT_d[_ts(oc, 128), b0 * N : (b0 + SPH) * N],
                        in_=ot[:, :],
                    )
    nc.finalize()
    return nc


_NC_CACHE = None
LAST_RESULT = None


def kernel(x, qkv_w, qkv_b, proj_w, proj_b, num_keep_node):
    global _NC_CACHE
    assert int(num_keep_node) == KEEP
    x = np.asarray(x, np.float32)
    scale = float(HD) ** -0.5

    wqkvT = np.ascontiguousarray(np.asarray(qkv_w, np.float32).T)  # [768, 2304]
    wqkvT[:, :C] = wqkvT[:, :C] * scale
    bqk = np.asarray(qkv_b, np.float32)[: 2 * C].copy()
    bqk[:C] *= scale
    bv = np.ascontiguousarray(np.asarray(qkv_b, np.float32)[2 * C :])
    wpT = np.ascontiguousarray(np.asarray(proj_w, np.float32).T)
    bp = np.ascontiguousarray(np.asarray(proj_b, np.float32))
    ones = np.ones(NPAIR, np.float32)

    if _NC_CACHE is None:
        _NC_CACHE = build_nc()
    nc = _NC_CACHE

    in_maps = []
    for c in range(NCORES):
        xs = x[c * B_CORE : (c + 1) * B_CORE].reshape(B_CORE * N, C)
        in_maps.append(
            {
                "xT": np.ascontiguousarray(xs.T),
                "wqkvT": wqkvT,
                "bqk": bqk,
                "bv": bv,
                "wpT": wpT,
                "bp": bp,
                "ones": ones,
            }
        )
    global LAST_RESULT
    LAST_RESULT = run_bass_kernel_spmd(
        nc,
        in_maps,
        core_ids=list(range(NCORES)),
        trace=bool(os.environ.get("KTRACE")),
        tmpdir=os.environ.get("KTRACE_DIR") or None,
    )
    res = LAST_RESULT.results

    outs, keeps, arts = [], [], []
    for r in res:
        outs.append(np.ascontiguousarray(r["outT"].reshape(C, B_CORE, N).transpose(1, 2, 0)))
        keeps.append(r["keep"][:, :, None])
        arts.append(np.ascontiguousarray(r["attn_rt_t"].transpose(0, 1, 3, 2)))
    return (
        np.concatenate(outs, 0),
        np.concatenate(keeps, 0),
        np.concatenate(arts, 0),
    )


# revision 13
# speedup vs baseline: 1.5524x; 1.4875x over previous
"""Trainium2 Bass kernel for nn_Attention_75806172775136 (topk_masking).

Data-parallel over 8 NeuronCores: 8 samples per core, weights replicated.
Reference computes, per sample: qkv proj -> attn logits -> CLS-token top-138
mask -> masked softmax -> attn @ v -> out proj; returns (out, keep_mask,
attn_rt).

Per-core pipeline (layouts transposed so contraction rides partitions; host
pre-transposes inputs / post-transposes outputs):
  1. qkT GEMM (fp32r, N=394 sample-pairs): psum += Wqk @ xT, bias via K=1
     matmul; copied out twice: bf16 (attention) + f32 k-chunks/q0 (exact cls)
  2. v GEMM (fp32r) in natural [n, o] layout -> bf16
  3. cls scores: fp32 matmuls q0 . kT  (exact top-k selection vs reference)
  4. top-138 via vector.max + match_replace (exact-K, matches jax top_k)
  5. S^T = kT.T @ qT per (sample, head) in bf16; raw logits -> attn_rt
  6. e^T = exp(S^T) (no max-shift: |S| < 40); AV with lhsT = [v*keep | keep]
     -> out2T rows 0-63 numerator, row 64 denominator
  7. normalize: reciprocal -> gpsimd partition_broadcast -> DVE mul -> A^T
  8. out^T = Wp @ A^T + b (fp32r, N=394)
"""

import os
import sys

sys.path.insert(0, "/opt/trn_rl_repo")

import numpy as np

import concourse.bass as bass
import concourse.bacc as bacc
import concourse.mybir as mybir
from concourse.tile import TileContext
from concourse.bass_utils import run_bass_kernel_spmd

F32 = mybir.dt.float32
F32R = mybir.dt.float32r
BF16 = mybir.dt.bfloat16
AF = mybir.ActivationFunctionType
OP = mybir.AluOpType

NCORES = 8
B_CORE = 8
N = 197
C = 768
H = 12
HD = 64
KEEP = 138
NEG = -1.0e9

SPH = 2              # samples per quarter
NH = SPH * N         # 394
NPAIR = 2 * N        # 394


def _ts(i, s):
    return slice(i * s, (i + 1) * s)


def build_nc():
    nc = bacc.Bacc("TRN2", target_bir_lowering=False, debug=False)

    xT_d = nc.declare_dram_parameter("xT", [C, B_CORE * N], F32, isOutput=False)
    wqkvT_d = nc.declare_dram_parameter("wqkvT", [C, 3 * C], F32, isOutput=False)
    bqk_d = nc.declare_dram_parameter("bqk", [2 * C], F32, isOutput=False)
    bv_d = nc.declare_dram_parameter("bv", [C], F32, isOutput=False)
    wpT_d = nc.declare_dram_parameter("wpT", [C, C], F32, isOutput=False)
    bp_d = nc.declare_dram_parameter("bp", [C], F32, isOutput=False)
    ones_d = nc.declare_dram_parameter("ones", [NPAIR], F32, isOutput=False)

    outT_d = nc.declare_dram_parameter("outT", [C, B_CORE * N], F32, isOutput=True)
    keep_d = nc.declare_dram_parameter("keep", [B_CORE, N], F32, isOutput=True)
    art_d = nc.declare_dram_parameter("attn_rt_t", [B_CORE, H, N, N], F32, isOutput=True)

    from contextlib import ExitStack

    with ExitStack() as ctx:
        tc = ctx.enter_context(TileContext(nc))
        ctx.enter_context(nc.allow_low_precision(reason="bf16/f32r compute by design"))
        cpool = ctx.enter_context(tc.tile_pool(name="const", bufs=1))
        wpool = ctx.enter_context(tc.tile_pool(name="wq", bufs=1))
        xpool = ctx.enter_context(tc.tile_pool(name="xh", bufs=2))
        qkpool = ctx.enter_context(tc.tile_pool(name="qk", bufs=2))
        vpool = ctx.enter_context(tc.tile_pool(name="vv", bufs=4))
        v2pool = ctx.enter_context(tc.tile_pool(name="v2", bufs=4))
        apool = ctx.enter_context(tc.tile_pool(name="att", bufs=4))
        npool = ctx.enter_context(tc.tile_pool(name="nrm", bufs=2))
        upool12 = ctx.enter_context(tc.tile_pool(name="usb12", bufs=12))
        scpool = ctx.enter_context(tc.tile_pool(name="sc", bufs=2))
        atpool = ctx.enter_context(tc.tile_pool(name="aT", bufs=2))
        opool = ctx.enter_context(tc.tile_pool(name="oT", bufs=2))
        ps_mm = ctx.enter_context(tc.tile_pool(name="ps_mm", bufs=2, space="PSUM"))
        ps_s = ctx.enter_context(tc.tile_pool(name="ps_s", bufs=3, space="PSUM"))
        ps_av = ctx.enter_context(tc.tile_pool(name="ps_av", bufs=3, space="PSUM"))

        # ---- weights / constants (persistent) ----
        w_sb = []
        for ci in range(6):
            t = wpool.tile([128, 3 * C], F32R, tag=f"wqkv{ci}", name=f"w{ci}")
            nc.sync.dma_start(out=t[:, :], in_=wqkvT_d[_ts(ci, 128), :].bitcast(F32R))
            w_sb.append(t)
        wp_sb = []
        for ci in range(6):
            t = wpool.tile([128, C], F32R, tag=f"wp{ci}", name=f"wp{ci}")
            nc.sync.dma_start(out=t[:, :], in_=wpT_d[_ts(ci, 128), :].bitcast(F32R))
            wp_sb.append(t)
        bqk_col = cpool.tile([128, 12], F32)
        nc.sync.dma_start(out=bqk_col[:, :], in_=bqk_d[:].rearrange("(c p) -> p c", p=128))
        bp_col = cpool.tile([128, 6], F32)
        nc.sync.dma_start(out=bp_col[:, :], in_=bp_d[:].rearrange("(c p) -> p c", p=128))
        bv_row = cpool.tile([1, C], F32R)
        nc.sync.dma_start(out=bv_row[:, :], in_=bv_d[:].rearrange("(a c) -> a c", a=1).bitcast(F32R))
        ones_r = cpool.tile([1, NPAIR], F32R)
        nc.sync.dma_start(out=ones_r[:, :], in_=ones_d[:].rearrange("(a c) -> a c", a=1).bitcast(F32R))

        state = {}  # per-quarter tiles

        def gemm_units(q):
            """Closure list: qkv oc-units, cls, v, topk, V2 for quarter q."""
            b0 = q * SPH
            st_ = {}
            units = []

            def u_load():
                xh = xpool.tile([128, 6, NH], F32R, tag="xT", name=f"xh{q}")
                for ci in range(6):
                    nc.sync.dma_start(
                        out=xh[:, ci, :],
                        in_=xT_d[_ts(ci, 128), b0 * N : (b0 + SPH) * N].bitcast(F32R),
                    )
                st_["xh"] = xh
                st_["qkb"] = qkpool.tile([128, 12, NH], BF16, tag="qkb", name=f"qkb{q}")
                st_["ktf"] = qkpool.tile([128, 6, NH], F32, tag="ktf", name=f"ktf{q}")
                st_["q0f"] = qkpool.tile([128, 6, SPH], F32, tag="q0f", name=f"q0f{q}")
            units.append(u_load)

            def mk_qkv(oc):
                def u():
                    xh, qkb, ktf, q0f = st_["xh"], st_["qkb"], st_["ktf"], st_["q0f"]
                    ps = ps_mm.tile([128, 512], F32, tag="mm", name=f"qkv{q}_{oc}")
                    for ci in range(6):
                        nc.tensor.matmul(
                            ps[:, :NH], lhsT=w_sb[ci][:, _ts(oc, 128)],
                            rhs=xh[:, ci, :], start=(ci == 0), stop=(ci == 5),
                        )
                    nc.vector.tensor_scalar(
                        out=qkb[:, oc, :], in0=ps[:, :NH],
                        scalar1=bqk_col[:, oc : oc + 1], scalar2=None, op0=OP.add,
                    )
                    if oc >= 6:
                        nc.vector.tensor_scalar(
                            out=ktf[:, oc - 6, :], in0=ps[:, :NH],
                            scalar1=bqk_col[:, oc : oc + 1], scalar2=None, op0=OP.add,
                        )
                    else:
                        for sb_ in range(SPH):
                            nc.vector.tensor_scalar(
                                out=q0f[:, oc, sb_ : sb_ + 1],
                                in0=ps[:, sb_ * N : sb_ * N + 1],
                                scalar1=bqk_col[:, oc : oc + 1], scalar2=None, op0=OP.add,
                            )
                return u
            units += [mk_qkv(oc) for oc in range(12)]

            def mk_cls(bl):
                def u():
                    ktf, q0f = st_["ktf"], st_["q0f"]
                    sc4 = st_.setdefault(
                        "sc4", scpool.tile([SPH, N], F32, tag="sc4", name=f"sc4{q}")
                    )
                    ps = ps_s.tile([1, N], F32, tag="sT", name=f"cls{q}_{bl}")
                    for ci in range(6):
                        nc.tensor.matmul(
                            ps[:, :], lhsT=q0f[:, ci, bl : bl + 1],
                            rhs=ktf[:, ci, bl * N : bl * N + N],
                            start=(ci == 0), stop=(ci == 5),
                        )
                    row = scpool.tile([1, N], F32, tag=f"scrow{bl}", name=f"scr{q}_{bl}")
                    nc.vector.tensor_copy(out=row[:, :], in_=ps[:, :])
                    nc.sync.dma_start(out=sc4[bl : bl + 1, :], in_=row[:, :])
                return u
            units += [mk_cls(bl) for bl in range(SPH)]

            def mk_v(bl, chk, r0, rn):
                def u():
                    xh = st_["xh"]
                    vts = st_.setdefault("v", {})
                    if bl not in vts:
                        vts[bl] = vpool.tile([128, 2, C], BF16, tag="v", name=f"v{q}_{bl}")
                    vt = vts[bl]
                    pss = [ps_mm.tile([128, 512], F32, tag="mm", name=f"v{q}_{bl}_{chk}_{_i}") for _i in range(2)]
                    for ci in range(6):
                        for ic, (c0, cn) in enumerate(((0, 512), (512, 256))):
                            nc.tensor.matmul(
                                pss[ic][:rn, :cn],
                                lhsT=xh[:, ci, bl * N + r0 : bl * N + r0 + rn],
                                rhs=w_sb[ci][:, 2 * C + c0 : 2 * C + c0 + cn],
                                start=(ci == 0), stop=False,
                            )
                    for ic, (c0, cn) in enumerate(((0, 512), (512, 256))):
                        nc.tensor.matmul(
                            pss[ic][:rn, :cn], lhsT=ones_r[:, :rn],
                            rhs=bv_row[:, c0 : c0 + cn], start=False, stop=True,
                        )
                        nc.scalar.copy(out=vt[:rn, chk, c0 : c0 + cn], in_=pss[ic][:rn, :cn])
                return u
            for bl in range(SPH):
                for chk, (r0, rn) in enumerate(((0, 128), (128, 69))):
                    units.append(mk_v(bl, chk, r0, rn))

            def u_topk():
                sc4 = st_["sc4"]
                work = scpool.tile([SPH, N], F32, tag="work", name=f"wk{q}")
                nc.vector.tensor_copy(out=work[:, :], in_=sc4[:, :])
                nc.vector.memset(work[:, 0:1], -2.0e9)
                m8 = scpool.tile([SPH, 8], F32, tag="m8", name=f"m8{q}")
                left = KEEP
                while left > 0:
                    nc.vector.max(out=m8[:, :], in_=work[:, :])
                    if left < 8:
                        nc.vector.memset(m8[:, left:], NEG)
                    nc.vector.match_replace(
                        out=work[:, :], in_to_replace=m8[:, :],
                        in_values=work[:, :], imm_value=NEG,
                    )
                    left -= 8
                keep4 = scpool.tile([SPH, N], F32, tag="keep4", name=f"kp{q}")
                nc.vector.tensor_tensor(out=keep4[:, :], in0=work[:, :], in1=sc4[:, :], op=OP.not_equal)
                nc.sync.dma_start(out=keep_d[q * SPH : (q + 1) * SPH, :], in_=keep4[:, :])
                kT1 = scpool.tile([128, SPH], F32, tag="kT1", name=f"kt1{q}")
                kT2 = scpool.tile([128, SPH], F32, tag="kT2", name=f"kt2{q}")
                for bl in range(SPH):
                    nc.sync.dma_start(out=kT1[:, bl : bl + 1], in_=keep4[bl : bl + 1, 0:128])
                    nc.sync.dma_start(out=kT2[0:69, bl : bl + 1], in_=keep4[bl : bl + 1, 128:197])
                st_["kT"] = (kT1, kT2)
            units.append(u_topk)

            def mk_v2(bl):
                def u():
                    kT1, kT2 = st_["kT"]
                    v2 = v2pool.tile([128, 2, H, HD + 1], BF16, tag="v2", name=f"v2_{q}_{bl}")
                    for chk, (kTc, rn) in enumerate(((kT1, 128), (kT2, 69))):
                        nc.vector.tensor_scalar(
                            out=v2[:rn, chk, :, 0:HD],
                            in0=st_["v"][bl][:rn, chk, :].rearrange("p (h d) -> p h d", h=H),
                            scalar1=kTc[:rn, bl : bl + 1], scalar2=None, op0=OP.mult,
                        )
                        nc.vector.tensor_copy(
                            out=v2[:rn, chk, :, HD],
                            in_=kTc[:rn, bl : bl + 1].to_broadcast([rn, H]),
                        )
                    st_.setdefault("v2", {})[bl] = v2
                return u
            units += [mk_v2(bl) for bl in range(SPH)]

            state[q] = st_
            return units

        def chain_closures(q):
            """24 attention chains + per-sample normalize tails for quarter q."""
            b0 = q * SPH
            st_ = state[q]
            out = []
            for bl in range(SPH):
                bb = b0 + bl

                def mk_begin(bl):
                    def u():
                        st_[f"D{bl}"] = npool.tile([H, N], BF16, tag="D", name=f"D{q}_{bl}")
                        st_[f"us{bl}"] = []
                    return u
                out.append(mk_begin(bl))

                def mk_chain(bl, bb, h):
                    def u():
                        qkb = st_["qkb"]
                        oc, po = h // 2, (h % 2) * HD
                        ps = ps_s.tile([128, NPAIR], F32, tag="sT", name=f"s{q}_{bl}_{h}")
                        nc.tensor.matmul(
                            ps[:, 0:N],
                            lhsT=qkb[po : po + HD, 6 + oc, bl * N : bl * N + 128],
                            rhs=qkb[po : po + HD, oc, bl * N : bl * N + N],
                            start=True, stop=True,
                        )
                        nc.tensor.matmul(
                            ps[0:69, N : 2 * N],
                            lhsT=qkb[po : po + HD, 6 + oc, bl * N + 128 : bl * N + N],
                            rhs=qkb[po : po + HD, oc, bl * N : bl * N + N],
                            start=True, stop=True,
                        )
                        st = apool.tile([128, NPAIR], F32, tag="sT_sb", name=f"st{q}_{bl}_{h}")
                        if h % 2 == 0:
                            nc.scalar.copy(out=st[:, :], in_=ps[:, :])
                        else:
                            nc.vector.tensor_copy(out=st[:, :], in_=ps[:, :])
                        nc.sync.dma_start(out=art_d[bb, h, 0:128, :], in_=st[:, 0:N])
                        nc.sync.dma_start(out=art_d[bb, h, 128:N, :], in_=st[0:69, N : 2 * N])
                        et = apool.tile([128, NPAIR], BF16, tag="eT", name=f"et{q}_{bl}_{h}")
                        nc.scalar.activation(out=et[:, :], in_=ps[:, :], func=AF.Exp)

                        ps_o = ps_av.tile([HD + 1, N], F32, tag="av", name=f"av{q}_{bl}_{h}")
                        nc.tensor.matmul(
                            ps_o[:, :], lhsT=st_["v2"][bl][0:128, 0, h, :],
                            rhs=et[:, 0:N], start=True, stop=False,
                        )
                        nc.tensor.matmul(
                            ps_o[:, :], lhsT=st_["v2"][bl][0:69, 1, h, :],
                            rhs=et[0:69, N : 2 * N], start=False, stop=True,
                        )
                        u65 = upool12.tile([HD + 1, N], BF16, tag="usb", name=f"u{q}_{bl}_{h}")
                        nc.vector.tensor_copy(out=u65[:, :], in_=ps_o[:, :])
                        nc.sync.dma_start(out=st_[f"D{bl}"][h : h + 1, :], in_=u65[HD : HD + 1, :])
                        st_[f"us{bl}"].append(u65)
                    return u
                out += [mk_chain(bl, bb, h) for h in range(H)]

                def mk_tail(bl):
                    def u():
                        aT = st_.setdefault(
                            "aT", atpool.tile([128, 6, NH], F32R, tag="aT", name=f"aT{q}")
                        )
                        R = npool.tile([H, N], BF16, tag="R", name=f"R{q}_{bl}")
                        nc.vector.reciprocal(out=R[:, :], in_=st_[f"D{bl}"][:, :])
                        for h in range(H):
                            oc, po = h // 2, (h % 2) * HD
                            rb = npool.tile([1, N], BF16, tag="rb", name=f"rb{q}_{bl}_{h}")
                            nc.sync.dma_start(out=rb[:, :], in_=R[h : h + 1, :])
                            bc = npool.tile([HD, N], BF16, tag="bc", name=f"bc{q}_{bl}_{h}")
                            nc.gpsimd.partition_broadcast(bc[:, :], rb[:, :])
                            nc.vector.tensor_tensor(
                                out=aT[po : po + HD, oc, bl * N : bl * N + N],
                                in0=st_[f"us{bl}"][h][0:HD, :],
                                in1=bc[:, :], op=OP.mult,
                            )
                    return u
                out.append(mk_tail(bl))
            return out

        def proj_units(q):
            b0 = q * SPH
            st_ = state[q]
            units = []

            def mk(oc):
                def u():
                    aT = st_["aT"]
                    ps = ps_mm.tile([128, 512], F32, tag="mm", name=f"pj{q}_{oc}")
                    for ci in range(6):
                        nc.tensor.matmul(
                            ps[:, :NPAIR], lhsT=wp_sb[ci][:, _ts(oc, 128)],
                            rhs=aT[:, ci, :], start=(ci == 0), stop=(ci == 5),
                        )
                    ot = opool.tile([128, NPAIR], F32, tag="oT", name=f"ot{q}_{oc}")
                    nc.vector.tensor_scalar(
                        out=ot[:, :], in0=ps[:, :NPAIR],
                        scalar1=bp_col[:, oc : oc + 1], scalar2=None, op0=OP.add,
                    )
                    nc.sync.dma_start(
                        out=outT_d[_ts(oc, 128), b0 * N : (b0 + SPH) * N], in_=ot[:, :],
                    )
                return u
            return [mk(oc) for oc in range(6)]

        # ---- software-pipelined emission ----
        pending = list(gemm_units(0))
        for u in pending:
            u()
        for q in range(4):
            nxt = list(gemm_units(q + 1)) if q < 3 else []
            chains = chain_closures(q)
            # interleave: spread nxt units across the chain emissions
            k = 0
            for j, ch in enumerate(chains):
                ch()
                want = (j + 1) * len(nxt) // len(chains)
                while k < want:
                    nxt[k]()
                    k += 1
            for u in proj_units(q):
                u()
    nc.finalize()
    return nc


_NC_CACHE = None
LAST_RESULT = None


def kernel(x, qkv_w, qkv_b, proj_w, proj_b, num_keep_node):
    global _NC_CACHE
    assert int(num_keep_node) == KEEP
    x = np.asarray(x, np.float32)
    scale = float(HD) ** -0.5

    wqkvT = np.ascontiguousarray(np.asarray(qkv_w, np.float32).T)  # [768, 2304]
    wqkvT[:, :C] = wqkvT[:, :C] * scale
    bqk = np.asarray(qkv_b, np.float32)[: 2 * C].copy()
    bqk[:C] *= scale
    bv = np.ascontiguousarray(np.asarray(qkv_b, np.float32)[2 * C :])
    wpT = np.ascontiguousarray(np.asarray(proj_w, np.float32).T)
    bp = np.ascontiguousarray(np.asarray(proj_b, np.float32))
    ones = np.ones(NPAIR, np.float32)

    if _NC_CACHE is None:
        _NC_CACHE = build_nc()
    nc = _NC_CACHE

    in_maps = []
    for c in range(NCORES):
        xs = x[c * B_CORE : (c + 1) * B_CORE].reshape(B_CORE * N, C)
        in_maps.append(
            {
                "xT": np.ascontiguousarray(xs.T),
                "wqkvT": wqkvT,
                "bqk": bqk,
                "bv": bv,
                "wpT": wpT,
                "bp": bp,
                "ones": ones,
            }
        )
    global LAST_RESULT
    LAST_RESULT = run_bass_kernel_spmd(
        nc,
        in_maps,
        core_ids=list(range(NCORES)),
        trace=bool(os.environ.get("KTRACE")),
        tmpdir=os.environ.get("KTRACE_DIR") or None,
    )
    res = LAST_RESULT.results

    outs, keeps, arts = [], [], []
    for r in res:
        outs.append(np.ascontiguousarray(r["outT"].reshape(C, B_CORE, N).transpose(1, 2, 0)))
        keeps.append(r["keep"][:, :, None])
        arts.append(np.ascontiguousarray(r["attn_rt_t"].transpose(0, 1, 3, 2)))
    return (
        np.concatenate(outs, 0),
        np.concatenate(keeps, 0),
        np.concatenate(arts, 0),
    )


# revision 16
# speedup vs baseline: 1.7063x; 1.0991x over previous
"""Trainium2 Bass kernel for nn_Attention_75806172775136 (topk_masking).

Data-parallel over 8 NeuronCores: 8 samples per core, weights replicated.
Reference computes, per sample: qkv proj -> attn logits -> CLS-token top-138
mask -> masked softmax -> attn @ v -> out proj; returns (out, keep_mask,
attn_rt).

Per-core pipeline (layouts transposed so contraction rides partitions; host
pre-transposes inputs / post-transposes outputs):
  1. qkT GEMM (fp32r, N=394 sample-pairs): psum += Wqk @ xT, bias via K=1
     matmul; copied out twice: bf16 (attention) + f32 k-chunks/q0 (exact cls)
  2. v GEMM (fp32r) in natural [n, o] layout -> bf16
  3. cls scores: fp32 matmuls q0 . kT  (exact top-k selection vs reference)
  4. top-138 via vector.max + match_replace (exact-K, matches jax top_k)
  5. S^T = kT.T @ qT per (sample, head) in bf16; raw logits -> attn_rt
  6. e^T = exp(S^T) (no max-shift: |S| < 40); AV with lhsT = [v*keep | keep]
     -> out2T rows 0-63 numerator, row 64 denominator
  7. normalize: reciprocal -> gpsimd partition_broadcast -> DVE mul -> A^T
  8. out^T = Wp @ A^T + b (fp32r, N=394)
"""

import os
import sys

sys.path.insert(0, "/opt/trn_rl_repo")

import numpy as np

import concourse.bass as bass
import concourse.bacc as bacc
import concourse.mybir as mybir
from concourse.tile import TileContext
from concourse.bass_utils import run_bass_kernel_spmd

F32 = mybir.dt.float32
F32R = mybir.dt.float32r
BF16 = mybir.dt.bfloat16
AF = mybir.ActivationFunctionType
OP = mybir.AluOpType

NCORES = 8
B_CORE = 8
N = 197
C = 768
H = 12
HD = 64
KEEP = 138
NEG = -1.0e9

SPH = 2              # samples per quarter
NH = SPH * N         # 394
NPAIR = 2 * N        # 394


def _ts(i, s):
    return slice(i * s, (i + 1) * s)


def build_nc():
    nc = bacc.Bacc("TRN2", target_bir_lowering=False, debug=False)

    xT_d = nc.declare_dram_parameter("xT", [C, B_CORE * N], F32, isOutput=False)
    wqkvT_d = nc.declare_dram_parameter("wqkvT", [C, 3 * C], F32, isOutput=False)
    bqk_d = nc.declare_dram_parameter("bqk", [2 * C], F32, isOutput=False)
    bv_d = nc.declare_dram_parameter("bv", [C], F32, isOutput=False)
    wpT_d = nc.declare_dram_parameter("wpT", [C, C], F32, isOutput=False)
    bp_d = nc.declare_dram_parameter("bp", [C], F32, isOutput=False)
    ones_d = nc.declare_dram_parameter("ones", [NPAIR], F32, isOutput=False)

    outT_d = nc.declare_dram_parameter("outT", [C, B_CORE * N], F32, isOutput=True)
    keep_d = nc.declare_dram_parameter("keep", [B_CORE, N], F32, isOutput=True)
    art_d = nc.declare_dram_parameter("attn_rt_t", [B_CORE, H, N, N], F32, isOutput=True)

    from contextlib import ExitStack

    with ExitStack() as ctx:
        tc = ctx.enter_context(TileContext(nc))
        ctx.enter_context(nc.allow_low_precision(reason="bf16/f32r compute by design"))
        cpool = ctx.enter_context(tc.tile_pool(name="const", bufs=1))
        wpool = ctx.enter_context(tc.tile_pool(name="wq", bufs=1))
        xpool = ctx.enter_context(tc.tile_pool(name="xh", bufs=2))
        qkpool = ctx.enter_context(tc.tile_pool(name="qk", bufs=2))
        ktpool = ctx.enter_context(tc.tile_pool(name="kt", bufs=1))
        vpool = ctx.enter_context(tc.tile_pool(name="vv", bufs=4))
        v2pool = ctx.enter_context(tc.tile_pool(name="v2", bufs=3))
        apool = ctx.enter_context(tc.tile_pool(name="att", bufs=3))
        npool = ctx.enter_context(tc.tile_pool(name="nrm", bufs=2))
        upool12 = ctx.enter_context(tc.tile_pool(name="usb12", bufs=3))
        scpool = ctx.enter_context(tc.tile_pool(name="sc", bufs=2))
        atpool = ctx.enter_context(tc.tile_pool(name="aT", bufs=2))
        opool = ctx.enter_context(tc.tile_pool(name="oT", bufs=2))
        ps_mm = ctx.enter_context(tc.tile_pool(name="ps_mm", bufs=2, space="PSUM"))
        ps_s = ctx.enter_context(tc.tile_pool(name="ps_s", bufs=3, space="PSUM"))
        ps_av = ctx.enter_context(tc.tile_pool(name="ps_av", bufs=3, space="PSUM"))

        # ---- weights / constants (persistent) ----
        w_sb = []
        for ci in range(6):
            t = wpool.tile([128, 3 * C], F32R, tag=f"wqkv{ci}", name=f"w{ci}")
            nc.sync.dma_start(out=t[:, :], in_=wqkvT_d[_ts(ci, 128), :].bitcast(F32R))
            w_sb.append(t)
        wp_sb = []
        for ci in range(6):
            t = wpool.tile([128, C], F32R, tag=f"wp{ci}", name=f"wp{ci}")
            nc.sync.dma_start(out=t[:, :], in_=wpT_d[_ts(ci, 128), :].bitcast(F32R))
            wp_sb.append(t)
        bqk_col = cpool.tile([128, 12], F32)
        nc.sync.dma_start(out=bqk_col[:, :], in_=bqk_d[:].rearrange("(c p) -> p c", p=128))
        bp_col = cpool.tile([128, 6], F32)
        nc.sync.dma_start(out=bp_col[:, :], in_=bp_d[:].rearrange("(c p) -> p c", p=128))
        bv_row = cpool.tile([1, C], F32R)
        nc.sync.dma_start(out=bv_row[:, :], in_=bv_d[:].rearrange("(a c) -> a c", a=1).bitcast(F32R))
        ones_r = cpool.tile([1, NPAIR], F32R)
        nc.sync.dma_start(out=ones_r[:, :], in_=ones_d[:].rearrange("(a c) -> a c", a=1).bitcast(F32R))

        state = {}  # per-quarter tiles

        def gemm_units(q):
            """Closure list: qkv oc-units, cls, v, topk, V2 for quarter q."""
            b0 = q * SPH
            st_ = {}
            units = []

            def u_load():
                xh = xpool.tile([128, 6, NH], F32R, tag="xT", name=f"xh{q}")
                for ci in range(6):
                    nc.sync.dma_start(
                        out=xh[:, ci, :],
                        in_=xT_d[_ts(ci, 128), b0 * N : (b0 + SPH) * N].bitcast(F32R),
                    )
                st_["xh"] = xh
                st_["qkb"] = qkpool.tile([128, 12, NH], BF16, tag="qkb", name=f"qkb{q}")
                st_["ktf"] = ktpool.tile([128, 6, NH], F32, tag="ktf", name=f"ktf{q}")
                st_["q0f"] = qkpool.tile([128, 6, SPH], F32, tag="q0f", name=f"q0f{q}")
            units.append(u_load)

            def mk_qkv(oc):
                def u():
                    xh, qkb, ktf, q0f = st_["xh"], st_["qkb"], st_["ktf"], st_["q0f"]
                    ps = ps_mm.tile([128, 512], F32, tag="mm", name=f"qkv{q}_{oc}")
                    for ci in range(6):
                        nc.tensor.matmul(
                            ps[:, :NH], lhsT=w_sb[ci][:, _ts(oc, 128)],
                            rhs=xh[:, ci, :], start=(ci == 0), stop=(ci == 5),
                        )
                    nc.vector.tensor_scalar(
                        out=qkb[:, oc, :], in0=ps[:, :NH],
                        scalar1=bqk_col[:, oc : oc + 1], scalar2=None, op0=OP.add,
                    )
                    if oc >= 6:
                        nc.vector.tensor_scalar(
                            out=ktf[:, oc - 6, :], in0=ps[:, :NH],
                            scalar1=bqk_col[:, oc : oc + 1], scalar2=None, op0=OP.add,
                        )
                    else:
                        nc.vector.tensor_scalar(
                            out=q0f[:, oc, :],
                            in0=ps[:, 0:NH:N],
                            scalar1=bqk_col[:, oc : oc + 1], scalar2=None, op0=OP.add,
                        )
                return u
            units += [mk_qkv(oc) for oc in range(12)]

            def mk_cls(bl):
                def u():
                    ktf, q0f = st_["ktf"], st_["q0f"]
                    sc4 = st_.setdefault(
                        "sc4", scpool.tile([SPH, N], F32, tag="sc4", name=f"sc4{q}")
                    )
                    ps = ps_s.tile([1, N], F32, tag="sT", name=f"cls{q}_{bl}")
                    for ci in range(6):
                        nc.tensor.matmul(
                            ps[:, :], lhsT=q0f[:, ci, bl : bl + 1],
                            rhs=ktf[:, ci, bl * N : bl * N + N],
                            start=(ci == 0), stop=(ci == 5),
                        )
                    row = scpool.tile([1, N], F32, tag=f"scrow{bl}", name=f"scr{q}_{bl}")
                    nc.vector.tensor_copy(out=row[:, :], in_=ps[:, :])
                    nc.gpsimd.dma_start(out=sc4[bl : bl + 1, :], in_=row[:, :])
                return u
            units += [mk_cls(bl) for bl in range(SPH)]

            def mk_v(bl, chk, r0, rn):
                def u():
                    xh = st_["xh"]
                    vts = st_.setdefault("v", {})
                    if bl not in vts:
                        vts[bl] = vpool.tile([128, 2, C], BF16, tag="v", name=f"v{q}_{bl}")
                    vt = vts[bl]
                    pss = [ps_mm.tile([128, 512], F32, tag="mm", name=f"v{q}_{bl}_{chk}_{_i}") for _i in range(2)]
                    for ci in range(6):
                        for ic, (c0, cn) in enumerate(((0, 512), (512, 256))):
                            nc.tensor.matmul(
                                pss[ic][:rn, :cn],
                                lhsT=xh[:, ci, bl * N + r0 : bl * N + r0 + rn],
                                rhs=w_sb[ci][:, 2 * C + c0 : 2 * C + c0 + cn],
                                start=(ci == 0), stop=False,
                            )
                    for ic, (c0, cn) in enumerate(((0, 512), (512, 256))):
                        nc.tensor.matmul(
                            pss[ic][:rn, :cn], lhsT=ones_r[:, :rn],
                            rhs=bv_row[:, c0 : c0 + cn], start=False, stop=True,
                        )
                        nc.scalar.copy(out=vt[:rn, chk, c0 : c0 + cn], in_=pss[ic][:rn, :cn])
                return u
            for bl in range(SPH):
                for chk, (r0, rn) in enumerate(((0, 128), (128, 69))):
                    units.append(mk_v(bl, chk, r0, rn))

            def u_topk():
                sc4 = st_["sc4"]
                work = scpool.tile([SPH, N], F32, tag="work", name=f"wk{q}")
                nc.vector.tensor_copy(out=work[:, :], in_=sc4[:, :])
                nc.vector.memset(work[:, 0:1], -2.0e9)
                m8 = scpool.tile([SPH, 8], F32, tag="m8", name=f"m8{q}")
                left = KEEP
                while left > 0:
                    nc.vector.max(out=m8[:, :], in_=work[:, :])
                    if left < 8:
                        nc.vector.memset(m8[:, left:], NEG)
                    nc.vector.match_replace(
                        out=work[:, :], in_to_replace=m8[:, :],
                        in_values=work[:, :], imm_value=NEG,
                    )
                    left -= 8
                keep4 = scpool.tile([SPH, N], F32, tag="keep4", name=f"kp{q}")
                nc.vector.tensor_tensor(out=keep4[:, :], in0=work[:, :], in1=sc4[:, :], op=OP.not_equal)
                nc.sync.dma_start(out=keep_d[q * SPH : (q + 1) * SPH, :], in_=keep4[:, :])
                kT1 = scpool.tile([128, SPH], F32, tag="kT1", name=f"kt1{q}")
                kT2 = scpool.tile([128, SPH], F32, tag="kT2", name=f"kt2{q}")
                for bl in range(SPH):
                    nc.gpsimd.dma_start(out=kT1[:, bl : bl + 1], in_=keep4[bl : bl + 1, 0:128])
                    nc.gpsimd.dma_start(out=kT2[0:69, bl : bl + 1], in_=keep4[bl : bl + 1, 128:197])
                st_["kT"] = (kT1, kT2)
            units.append(u_topk)

            def mk_v2(bl):
                def u():
                    kT1, kT2 = st_["kT"]
                    v2 = v2pool.tile([128, 2, H, HD + 1], BF16, tag="v2", name=f"v2_{q}_{bl}")
                    for chk, (kTc, rn) in enumerate(((kT1, 128), (kT2, 69))):
                        nc.vector.tensor_scalar(
                            out=v2[:rn, chk, :, 0:HD],
                            in0=st_["v"][bl][:rn, chk, :].rearrange("p (h d) -> p h d", h=H),
                            scalar1=kTc[:rn, bl : bl + 1], scalar2=None, op0=OP.mult,
                        )
                        nc.vector.tensor_copy(
                            out=v2[:rn, chk, :, HD],
                            in_=kTc[:rn, bl : bl + 1].to_broadcast([rn, H]),
                        )
                    st_.setdefault("v2", {})[bl] = v2
                return u
            units += [mk_v2(bl) for bl in range(SPH)]

            state[q] = st_
            return units

        def chain_closures(q):
            """24 attention chains + per-sample normalize tails for quarter q."""
            b0 = q * SPH
            st_ = state[q]
            out = []
            for bl in range(SPH):
                bb = b0 + bl

                def mk_begin(bl):
                    def u():
                        st_[f"D{bl}"] = npool.tile([H, N], BF16, tag="D", name=f"D{q}_{bl}")
                        st_[f"U{bl}"] = upool12.tile([HD + 1, H, N], BF16, tag="usb", name=f"U{q}_{bl}")
                    return u
                out.append(mk_begin(bl))

                def mk_chain(bl, bb, h):
                    def u():
                        qkb = st_["qkb"]
                        oc, po = h // 2, (h % 2) * HD
                        ps = ps_s.tile([128, NPAIR], F32, tag="sT", name=f"s{q}_{bl}_{h}")
                        nc.tensor.matmul(
                            ps[:, 0:N],
                            lhsT=qkb[po : po + HD, 6 + oc, bl * N : bl * N + 128],
                            rhs=qkb[po : po + HD, oc, bl * N : bl * N + N],
                            start=True, stop=True,
                        )
                        nc.tensor.matmul(
                            ps[0:69, N : 2 * N],
                            lhsT=qkb[po : po + HD, 6 + oc, bl * N + 128 : bl * N + N],
                            rhs=qkb[po : po + HD, oc, bl * N : bl * N + N],
                            start=True, stop=True,
                        )
                        st = apool.tile([128, NPAIR], F32, tag="sT_sb", name=f"st{q}_{bl}_{h}")
                        if h % 2 == 0:
                            nc.scalar.copy(out=st[:, :], in_=ps[:, :])
                        else:
                            nc.vector.tensor_copy(out=st[:, :], in_=ps[:, :])
                        nc.sync.dma_start(out=art_d[bb, h, 0:128, :], in_=st[:, 0:N])
                        nc.gpsimd.dma_start(out=art_d[bb, h, 128:N, :], in_=st[0:69, N : 2 * N])
                        et = apool.tile([128, NPAIR], BF16, tag="eT", name=f"et{q}_{bl}_{h}")
                        nc.scalar.activation(out=et[:, :], in_=ps[:, :], func=AF.Exp)

                        ps_o = ps_av.tile([HD + 1, N], F32, tag="av", name=f"av{q}_{bl}_{h}")
                        nc.tensor.matmul(
                            ps_o[:, :], lhsT=st_["v2"][bl][0:128, 0, h, :],
                            rhs=et[:, 0:N], start=True, stop=False,
                        )
                        nc.tensor.matmul(
                            ps_o[:, :], lhsT=st_["v2"][bl][0:69, 1, h, :],
                            rhs=et[0:69, N : 2 * N], start=False, stop=True,
                        )
                        nc.vector.tensor_copy(out=st_[f"U{bl}"][:, h, :], in_=ps_o[:, :])
                    return u
                out += [mk_chain(bl, bb, h) for h in range(H)]

                def mk_tail(bl):
                    def u():
                        aT = st_.setdefault(
                            "aT", atpool.tile([128, 6, NH], F32R, tag="aT", name=f"aT{q}")
                        )
                        U = st_[f"U{bl}"]
                        D = st_[f"D{bl}"]
                        nc.gpsimd.dma_start(out=D[:, :], in_=U[HD : HD + 1, :, :])
                        R = npool.tile([H, N], BF16, tag="R", name=f"R{q}_{bl}")
                        nc.vector.reciprocal(out=R[:, :], in_=D[:, :])
                        RB = npool.tile([1, H, N], BF16, tag="rb", name=f"rb{q}_{bl}")
                        nc.gpsimd.dma_start(out=RB[:, :, :], in_=R[:, :])
                        for h in range(H):
                            oc, po = h // 2, (h % 2) * HD
                            bc = npool.tile([HD, N], BF16, tag="bc", name=f"bc{q}_{bl}_{h}")
                            nc.gpsimd.partition_broadcast(bc[:, :], RB[0:1, h, :])
                            nc.vector.tensor_tensor(
                                out=aT[po : po + HD, oc, bl * N : bl * N + N],
                                in0=U[0:HD, h, :],
                                in1=bc[:, :], op=OP.mult,
                            )
                    return u
                out.append(mk_tail(bl))
            return out

        def proj_units(q):
            b0 = q * SPH
            st_ = state[q]
            units = []

            def mk(oc):
                def u():
                    aT = st_["aT"]
                    ps = ps_mm.tile([128, 512], F32, tag="mm", name=f"pj{q}_{oc}")
                    for ci in range(6):
                        nc.tensor.matmul(
                            ps[:, :NPAIR], lhsT=wp_sb[ci][:, _ts(oc, 128)],
                            rhs=aT[:, ci, :], start=(ci == 0), stop=(ci == 5),
                        )
                    ot = opool.tile([128, NPAIR], F32, tag="oT", name=f"ot{q}_{oc}")
                    nc.vector.tensor_scalar(
                        out=ot[:, :], in0=ps[:, :NPAIR],
                        scalar1=bp_col[:, oc : oc + 1], scalar2=None, op0=OP.add,
                    )
                    nc.sync.dma_start(
                        out=outT_d[_ts(oc, 128), b0 * N : (b0 + SPH) * N], in_=ot[:, :],
                    )
                return u
            return [mk(oc) for oc in range(6)]

        # ---- software-pipelined emission ----
        pending = list(gemm_units(0))
        for u in pending:
            u()
        for q in range(4):
            nxt = list(gemm_units(q + 1)) if q < 3 else []
            chains = chain_closures(q)
            # interleave: spread nxt units across the chain emissions
            k = 0
            for j, ch in enumerate(chains):
                ch()
                want = (j + 1) * len(nxt) // len(chains)
                while k < want:
                    nxt[k]()
                    k += 1
            for u in proj_units(q):
                u()
    nc.finalize()
    return nc


_NC_CACHE = None
LAST_RESULT = None


def kernel(x, qkv_w, qkv_b, proj_w, proj_b, num_keep_node):
    global _NC_CACHE
    assert int(num_keep_node) == KEEP
    x = np.asarray(x, np.float32)
    scale = float(HD) ** -0.5

    wqkvT = np.ascontiguousarray(np.asarray(qkv_w, np.float32).T)  # [768, 2304]
    wqkvT[:, :C] = wqkvT[:, :C] * scale
    bqk = np.asarray(qkv_b, np.float32)[: 2 * C].copy()
    bqk[:C] *= scale
    bv = np.ascontiguousarray(np.asarray(qkv_b, np.float32)[2 * C :])
    wpT = np.ascontiguousarray(np.asarray(proj_w, np.float32).T)
    bp = np.ascontiguousarray(np.asarray(proj_b, np.float32))
    ones = np.ones(NPAIR, np.float32)

    if _NC_CACHE is None:
        _NC_CACHE = build_nc()
    nc = _NC_CACHE

    in_maps = []
    for c in range(NCORES):
        xs = x[c * B_CORE : (c + 1) * B_CORE].reshape(B_CORE * N, C)
        in_maps.append(
            {
                "xT": np.ascontiguousarray(xs.T),
                "wqkvT": wqkvT,
                "bqk": bqk,
                "bv": bv,
                "wpT": wpT,
                "bp": bp,
                "ones": ones,
            }
        )
    global LAST_RESULT
    LAST_RESULT = run_bass_kernel_spmd(
        nc,
        in_maps,
        core_ids=list(range(NCORES)),
        trace=bool(os.environ.get("KTRACE")),
        tmpdir=os.environ.get("KTRACE_DIR") or None,
    )
    res = LAST_RESULT.results

    outs, keeps, arts = [], [], []
    for r in res:
        outs.append(np.ascontiguousarray(r["outT"].reshape(C, B_CORE, N).transpose(1, 2, 0)))
        keeps.append(r["keep"][:, :, None])
        arts.append(np.ascontiguousarray(r["attn_rt_t"].transpose(0, 1, 3, 2)))
    return (
        np.concatenate(outs, 0),
        np.concatenate(keeps, 0),
        np.concatenate(arts, 0),
    )
